# revision 11
# baseline (speedup 1.0000x reference)
"""Banded-causal complex attention on 8 Trainium2 NeuronCores.

Strategy: data-parallel over batch (B=8 -> 1 batch per core), bf16 compute
with fp32 PSUM accumulation (validated ~3e-3 rel err vs the 2e-2 gate).

Per core:
  - Q is packed [Wqr|Wqi]*scale^2*temp, K is packed [Wkr|-Wki]: the complex
    score real part (qr.kr - qi.ki)*scale*temp becomes ONE K=128 matmul.
  - bf16 halves DMA bytes (per-core DMA is ~358 GB/s total, so the 3.3MB
    input floor is ~9.3us) and enables FWL fast weight loads on the PE.
  - DMA triggers cost ~650ns on the issuing engine, so weights/consts/pos
    are merged into few tensors; bulk input rides the sync queue, weights
    the gpsimd queue, and the ACT engine issues no DMA at all.
  - piece 0 of x^T arrives as 4 chunk-sized DMAs so the first projection
    matmul can start as soon as ~one chunk + weights have landed; a short
    junk-matmul warmup before that keeps the PE HAM clock gate ramping.
  - scores are computed transposed: sT_kb[key c, query r] covers the two
    query blocks (kb, kb+1) that attend key block kb, one N=256 matmul each.
  - band+causal masking: triangular affine_select on GpSimd for the diag
    half, a mask multiply on DVE for the off-diag half of exp(sT).
  - v_aug[key, 0:64|1.0] is produced by ONE regular matmul per key block:
    stationary = [vT ; ones-row] (65 x 128), moving = const [65, 66] that
    is [I64; bv^T] with a ones column -- folds the transpose, the V bias
    add and the rowsum-rider in a single cheap N=66 matmul.
  - softmax skips the max-subtraction (scores are O(5); masked entries are
    exactly zero); row sums ride along as the ones column.
  - outputs are normalized per block as soon as attended (DVE reciprocal +
    tensor_scalar mul) and DMA'd out every 4 blocks, so the kernel tail is
    only the last 4-block DMA + queue drains.
"""

import numpy as np

B, S, D, KD = 8, 2048, 512, 64
P = 128              # partition size / query block
NB = S // P          # 16 query/key blocks
DCH = D // P         # 4 contraction chunks
NCH = 4              # column pieces
NSL = S // NCH       # 512 columns per piece
NCORES = 8
NWARM = 9            # junk warmup matmuls (N=256)

_CACHE = {}
TRACE_KWARGS = {}    # test harness may set e.g. {"trace": True, "tmpdir": ...}


def _build_nc():
    import concourse.bacc as bacc
    import concourse.tile as tile
    import concourse.mybir as mybir
    from concourse.bass import ts

    f32 = mybir.dt.float32
    bf = mybir.dt.bfloat16
    nc = bacc.Bacc(None)

    xtr = nc.declare_dram_parameter("xtr", [NCH, P, DCH, NSL], bf, isOutput=False)
    wall = nc.declare_dram_parameter("wall", [P, DCH, 2 * P + KD], bf, isOutput=False)
    pqk = nc.declare_dram_parameter("pqk", [P, 2, S], bf, isOutput=False)
    # cst: cols 0:128 = off-diag mask [key c, query r] (c>=r);
    #      cols 128:194 rows 0:65 = vgen moving const [I64;bv^T | ones col]
    cst = nc.declare_dram_parameter("cst", [P, P + KD + 2], bf, isOutput=False)
    out = nc.declare_dram_parameter("out", [S, KD], f32, isOutput=True)

    with tile.TileContext(nc) as tc:
        with (
            tc.tile_pool(name="consts", bufs=1) as consts,
            tc.tile_pool(name="persist", bufs=1) as persist,
            tc.tile_pool(name="work", bufs=6) as work,
            tc.tile_pool(name="ps_proj", bufs=2, space="PSUM") as ps_proj,
            tc.tile_pool(name="ps_s", bufs=2, space="PSUM") as ps_s,
            tc.tile_pool(name="ps_small", bufs=2, space="PSUM") as ps_small,
        ):
            # warm the ACT exp table before it's on the critical path
            dummy = consts.tile([P, 2], f32)
            nc.vector.memset(dummy, 0.0)
            nc.scalar.activation(
                out=dummy, in_=dummy, func=mybir.ActivationFunctionType.Exp
            )

            # ALL bulk input rides ONE queue (sync) in strict priority
            # order; DMA trigger instructions cost ~650ns each on the
            # issuing engine, so order = arrival order. Consts ride the
            # gpsimd queue (tiny, lands first).
            wall_sb = consts.tile([P, DCH, 2 * P + KD], bf)
            nc.sync.dma_start(out=wall_sb, in_=wall[:])
            cst_sb = consts.tile([P, P + KD + 2], bf)
            nc.gpsimd.dma_start(out=cst_sb, in_=cst[:])
            pqk_sb = persist.tile([P, 2, S], bf)

            msk_sb = cst_sb[:, 0:P]
            mov_sb = cst_sb[0 : KD + 1, P : P + KD + 2]
            wq_sb = wall_sb[:, :, 0:P]
            wk_sb = wall_sb[:, :, P : 2 * P]
            wv_sb = wall_sb[:, :, 2 * P : 2 * P + KD]

            # piece-0 x^T in two halves so projections start sooner; pos
            # tables trail their piece (the pos-dependent work is cheap)
            xT_sb = persist.tile([P, NCH, DCH, NSL], bf)
            nc.sync.dma_start(out=xT_sb[:, 0, 0:2], in_=xtr[0, :, 0:2])
            nc.sync.dma_start(out=xT_sb[:, 0, 2:4], in_=xtr[0, :, 2:4])
            nc.sync.dma_start(out=pqk_sb[:, :, 0:NSL], in_=pqk[:, :, 0:NSL])
            for n in range(1, NCH):
                sl = slice(n * NSL, (n + 1) * NSL)
                nc.sync.dma_start(out=xT_sb[:, n], in_=xtr[n])
                nc.sync.dma_start(out=pqk_sb[:, :, sl], in_=pqk[:, :, sl])

            # junk matmuls: hold the PE busy (HAM ramp) while DMA lands
            zjunk = consts.tile([P, 2 * P], bf)
            nc.vector.memset(zjunk, 0.0)
            ps_dum = ps_proj.tile([P, 2 * P], f32, tag="ps", name="ps_dum")
            for _ in range(NWARM):
                nc.tensor.matmul(
                    ps_dum, zjunk[:, 0:P], zjunk, start=True, stop=True
                )

            # qT padded by one block so every sT matmul is a uniform N=256
            qT_sb = persist.tile([P, S + P], bf)
            kT_sb = persist.tile([P, S], bf)
            nc.vector.memset(qT_sb[:, S : S + P], 0.0)

            # vT rows 0:64 = raw v projection; row 64 = ones (rowsum rider)
            vT_sb = persist.tile([KD + 1, S], bf)
            nc.vector.memset(vT_sb[KD : KD + 1, :], 1.0)

            # v_aug[key, block, 0:64] = v + bv; col 64 = 1.0; col 65 pad
            v_aug = persist.tile([P, NB, KD + 2], bf)

            out_all = persist.tile([P, NB, KD], f32)
            recip_sb = persist.tile([P, NB, 1], f32)
            out_r = out.rearrange("(q r) k -> r q k", r=P)

            def proj_piece(n):
                sl = slice(n * NSL, (n + 1) * NSL)
                for grp in range(3):  # 0=q, 1=k, 2=v
                    w_g = (wq_sb, wk_sb, wv_sb)[grp]
                    m = P if grp < 2 else KD
                    ps = ps_proj.tile([m, NSL], f32, tag="ps", name="ps")
                    for c in range(DCH):
                        nc.tensor.matmul(
                            ps,
                            w_g[:, c, :m],
                            xT_sb[:, n, c, :],
                            start=(c == 0),
                            stop=(c == DCH - 1),
                        )
                    if grp == 0:
                        nc.vector.tensor_add(qT_sb[:, sl], ps, pqk_sb[:, 0, sl])
                    elif grp == 1:
                        nc.vector.tensor_add(kT_sb[:, sl], ps, pqk_sb[:, 1, sl])
                    else:
                        nc.scalar.copy(vT_sb[0:KD, sl], ps)

            def vgen(t):
                # [128 keys, 66] = vT_aug_block.T @ [I64;bv^T|ones-col]
                g = t // 4
                if t % 4 == 0:
                    v_group[g] = ps_small.tile(
                        [P, 4, KD + 2], f32, tag="small", name="vp4"
                    )
                nc.tensor.matmul(
                    v_group[g][:, t % 4, :], vT_sb[:, ts(t, P)], mov_sb,
                    start=True, stop=True,
                )
                if t % 4 == 3:
                    nc.vector.tensor_copy(
                        v_aug[:, 4 * g : 4 * g + 4, :], v_group.pop(g)
                    )

            p_tiles = {}
            o_group = {}
            v_group = {}

            def score_pair(kbs):
                # paired blocks share one PSUM tile / one exp / one affine /
                # one mask-mul, halving the fixed per-op overheads that
                # otherwise pace the score->exp->mask->attend chain
                s_ps = ps_s.tile([P, 2, 2 * P], f32, tag="s", name="s_ps")
                for j, kb in enumerate(kbs):
                    nc.tensor.matmul(
                        s_ps[:, j, :],
                        kT_sb[:, ts(kb, P)],
                        qT_sb[:, kb * P : kb * P + 2 * P],
                        start=True, stop=True,
                    )
                nb = len(kbs)
                p_sb = work.tile([P, 2, 2, P], bf, tag="p_sb", name="p_sb")
                nc.scalar.activation(
                    out=p_sb[:, 0:nb],
                    in_=s_ps[:, 0:nb].rearrange("c b (h r) -> c b h r", h=2),
                    func=mybir.ActivationFunctionType.Exp,
                )
                # band+causal: halves 0 keep keys c <= r (diag blocks) on
                # gpsimd, halves 1 keep c >= r (off-diag) on DVE/gpsimd
                nc.gpsimd.affine_select(
                    out=p_sb[:, 0:nb, 0, :], in_=p_sb[:, 0:nb, 0, :],
                    pattern=[[0, nb], [1, P]],
                    compare_op=mybir.AluOpType.is_ge,
                    fill=0.0, base=0, channel_multiplier=-1,
                )
                eng = nc.vector if kbs[0] % 4 < 2 else nc.gpsimd
                eng.tensor_mul(
                    p_sb[:, 0:nb, 1, :], p_sb[:, 0:nb, 1, :],
                    msk_sb.rearrange("c (b r) -> c b r", b=1).to_broadcast(
                        (P, nb, P)
                    ),
                )
                for j, kb in enumerate(kbs):
                    p_tiles[kb] = p_sb[:, j]

            def attend(qb):
                g = qb // 4
                if qb % 4 == 0:
                    o_group[g] = ps_small.tile(
                        [P, 4, KD + 2], f32, tag="og", name="o4", bufs=2
                    )
                o_ps = o_group[g][:, qb % 4, :]
                halves = [(p_tiles[qb], 0, qb)]
                if qb > 0:
                    halves.insert(0, (p_tiles[qb - 1], 1, qb - 1))
                for i, (pt, h, kb2) in enumerate(halves):
                    nc.tensor.matmul(
                        o_ps,
                        pt[:, h, :],
                        v_aug[:, kb2, :],
                        start=(i == 0),
                        stop=(i == len(halves) - 1),
                    )
                if qb > 0:
                    p_tiles.pop(qb - 1, None)
                g4 = NB // 4 - 1
                if g == g4 and qb % 2 == 1:
                    # final group: normalize + DMA per 2 blocks on separate
                    # queues so the last transfer is small and completions
                    # overlap the teardown
                    h = (qb - 1) // 2 - 2 * g4  # 0 or 1 within final group
                    o4 = o_group[g]
                    sl2 = slice(4 * g + 2 * h, 4 * g + 2 * h + 2)
                    r2 = recip_sb[:, sl2, :]
                    nc.vector.reciprocal(r2, o4[:, 2 * h : 2 * h + 2, KD : KD + 1])
                    nc.vector.tensor_mul(
                        out_all[:, sl2, :],
                        o4[:, 2 * h : 2 * h + 2, 0:KD],
                        r2.to_broadcast((P, 2, KD)),
                    )
                    eng = nc.sync if h == 0 else nc.scalar
                    eng.dma_start(out=out_r[:, sl2, :], in_=out_all[:, sl2, :])
                    if qb == NB - 1:
                        o_group.pop(g)
                elif qb % 4 == 3:
                    # batched normalize of the 4-block group, then DMA out
                    o4 = o_group.pop(g)
                    r4 = recip_sb[:, 4 * g : 4 * g + 4, :]
                    nc.vector.reciprocal(r4, o4[:, :, KD : KD + 1])
                    nc.vector.tensor_mul(
                        out_all[:, 4 * g : 4 * g + 4, :],
                        o4[:, :, 0:KD],
                        r4.to_broadcast((P, 4, KD)),
                    )
                    nc.sync.dma_start(
                        out=out_r[:, 4 * g : 4 * g + 4, :],
                        in_=out_all[:, 4 * g : 4 * g + 4, :],
                    )

            def junk(k):
                # filler matmuls at known DMA-wait points: keep the HAM
                # busy-streak alive without delaying real work much
                for _ in range(k):
                    nc.tensor.matmul(
                        ps_dum, zjunk[:, 0:P], zjunk, start=True, stop=True
                    )

            # ---- software-pipelined schedule over the 4 column pieces
            # score pairs: {0}, {1,2}, {3,4}, ..., {13,14}, {15}
            pairs = [[0]] + [[2 * g - 1, 2 * g] for g in range(1, NB // 2)] + [[NB - 1]]
            scored = 0   # next pair index
            attended = 0
            for n in range(NCH):
                if n in (1, 2):
                    junk(2)
                proj_piece(n)
                hi = 2 * n + 1 if n < NCH - 1 else len(pairs) - 1
                score_pair(pairs[scored]); scored += 1
                for t in range(4 * n, 4 * (n + 1)):
                    vgen(t)
                while scored <= hi:
                    done = pairs[scored - 1][-1]  # blocks masked so far
                    while done - attended > 2:
                        attend(attended)
                        attended += 1
                    score_pair(pairs[scored]); scored += 1
            while attended < NB:
                attend(attended)
                attended += 1

    nc.finalize()
    return nc


def _prep_core_inputs(inputs):
    import ml_dtypes

    bfnp = ml_dtypes.bfloat16
    g = lambda k: np.asarray(inputs[k], dtype=np.float32)
    x = g("x")
    scale = 1.0 / np.sqrt(np.float32(KD))
    temp = float(np.asarray(inputs["temperature"]).reshape(-1)[0])
    alpha = scale * temp  # folded (softmax temp) * (score scale)

    wq = np.concatenate([g("Wqr"), g("Wqi")], axis=1) * (scale * alpha)
    pq = np.concatenate(
        [
            g("pos_qr") * alpha + g("bqr") * (scale * alpha),
            g("pos_qi") * alpha + g("bqi") * (scale * alpha),
        ],
        axis=1,
    ).T  # [128, S]
    wk = np.concatenate([g("Wkr"), -g("Wki")], axis=1)
    pk = np.concatenate(
        [g("pos_kr") + g("bkr"), -(g("pos_ki") + g("bki"))], axis=1
    ).T
    wv = g("Wv")
    bv = g("bv").reshape(KD)

    # merged weights [128, DCH, 320]: [wq | wk | wv]
    pe_pack = lambda w: np.ascontiguousarray(
        w.reshape(DCH, P, w.shape[1]).transpose(1, 0, 2)
    )
    wall = np.concatenate(
        [pe_pack(wq), pe_pack(wk), pe_pack(wv)], axis=2
    )

    # merged consts [128, 128+66]: off-diag mask | vgen moving const
    cc, rr = np.meshgrid(np.arange(P), np.arange(P), indexing="ij")
    cst = np.zeros((P, P + KD + 2), dtype=np.float32)
    cst[:, :P] = (cc >= rr).astype(np.float32)
    cst[:KD, P : P + KD] = np.eye(KD)
    cst[KD, P : P + KD] = bv
    cst[KD, P + KD] = 1.0

    shared = {
        "wall": np.ascontiguousarray(wall).astype(bfnp),
        "pqk": np.ascontiguousarray(np.stack([pq, pk], axis=1)).astype(bfnp),
        "cst": cst.astype(bfnp),
    }
    in_maps = []
    for b in range(NCORES):
        m = dict(shared)
        # xtr[n, p, c, j] = x[b].T[c*128+p, n*512+j]
        xT_b = np.ascontiguousarray(x[b].T)
        m["xtr"] = np.ascontiguousarray(
            xT_b.reshape(DCH, P, NCH, NSL).transpose(2, 1, 0, 3)
        ).astype(bfnp)
        in_maps.append(m)
    return in_maps


def kernel(**inputs):
    from concourse.bass_utils import run_bass_kernel_spmd

    nc = _CACHE.get("nc")
    if nc is None:
        nc = _CACHE["nc"] = _build_nc()
    in_maps = _prep_core_inputs(inputs)
    res = run_bass_kernel_spmd(
        nc, in_maps, core_ids=list(range(NCORES)), **TRACE_KWARGS
    )
    _CACHE["last_result"] = res
    return np.stack([res.results[b]["out"] for b in range(NCORES)], axis=0)


# revision 12
# speedup vs baseline: 1.1750x; 1.1750x over previous
"""Banded-causal complex attention on 8 Trainium2 NeuronCores.

Strategy: data-parallel over batch (B=8 -> 1 batch per core), bf16 compute
with fp32 PSUM accumulation (validated ~3e-3 rel err vs the 2e-2 gate).

Per core:
  - Q is packed [Wqr|Wqi]*scale^2*temp, K is packed [Wkr|-Wki]: the complex
    score real part (qr.kr - qi.ki)*scale*temp becomes ONE K=128 matmul.
  - bf16 halves DMA bytes (per-core DMA is ~358 GB/s total, so the 3.3MB
    input floor is ~9.3us) and enables FWL fast weight loads on the PE.
  - DMA triggers cost ~650ns on the issuing engine, so weights/consts/pos
    are merged into few tensors; bulk input rides the sync queue, weights
    the gpsimd queue, and the ACT engine issues no DMA at all.
  - piece 0 of x^T arrives as 4 chunk-sized DMAs so the first projection
    matmul can start as soon as ~one chunk + weights have landed; a short
    junk-matmul warmup before that keeps the PE HAM clock gate ramping.
  - scores are computed transposed: sT_kb[key c, query r] covers the two
    query blocks (kb, kb+1) that attend key block kb, one N=256 matmul each.
  - band+causal masking: triangular affine_select on GpSimd for the diag
    half, a mask multiply on DVE for the off-diag half of exp(sT).
  - v_aug[key, 0:64|1.0] is produced by ONE regular matmul per key block:
    stationary = [vT ; ones-row] (65 x 128), moving = const [65, 66] that
    is [I64; bv^T] with a ones column -- folds the transpose, the V bias
    add and the rowsum-rider in a single cheap N=66 matmul.
  - softmax skips the max-subtraction (scores are O(5); masked entries are
    exactly zero); row sums ride along as the ones column.
  - outputs are normalized per block as soon as attended (DVE reciprocal +
    tensor_scalar mul) and DMA'd out every 4 blocks, so the kernel tail is
    only the last 4-block DMA + queue drains.
"""

import numpy as np

B, S, D, KD = 8, 2048, 512, 64
P = 128              # partition size / query block
NB = S // P          # 16 query/key blocks
DCH = D // P         # 4 contraction chunks
NCH = 4              # column pieces
NSL = S // NCH       # 512 columns per piece
NCORES = 8
NWARM = 13           # junk warmup matmuls (N=256)

_CACHE = {}
TRACE_KWARGS = {}    # test harness may set e.g. {"trace": True, "tmpdir": ...}


def _build_nc():
    import concourse.bacc as bacc
    import concourse.tile as tile
    import concourse.mybir as mybir
    from concourse.bass import ts

    f32 = mybir.dt.float32
    bf = mybir.dt.bfloat16
    nc = bacc.Bacc(None)

    xtr = nc.declare_dram_parameter("xtr", [NCH, P, DCH, NSL], bf, isOutput=False)
    wall = nc.declare_dram_parameter("wall", [P, DCH, 2 * P + KD], bf, isOutput=False)
    pqk = nc.declare_dram_parameter("pqk", [P, 2, S], bf, isOutput=False)
    # cst: cols 0:128 = off-diag mask [key c, query r] (c>=r);
    #      cols 128:194 rows 0:65 = vgen moving const [I64;bv^T | ones col]
    cst = nc.declare_dram_parameter("cst", [P, P + KD + 2], bf, isOutput=False)
    out = nc.declare_dram_parameter("out", [S, KD], f32, isOutput=True)

    with tile.TileContext(nc) as tc:
        with (
            tc.tile_pool(name="consts", bufs=1) as consts,
            tc.tile_pool(name="persist", bufs=1) as persist,
            tc.tile_pool(name="work", bufs=6) as work,
            tc.tile_pool(name="ps_proj", bufs=2, space="PSUM") as ps_proj,
            tc.tile_pool(name="ps_s", bufs=2, space="PSUM") as ps_s,
            tc.tile_pool(name="ps_small", bufs=2, space="PSUM") as ps_small,
        ):
            # warm the ACT exp table before it's on the critical path
            dummy = consts.tile([P, 2], f32)
            nc.vector.memset(dummy, 0.0)
            nc.scalar.activation(
                out=dummy, in_=dummy, func=mybir.ActivationFunctionType.Exp
            )

            # ALL bulk input rides ONE queue (sync) in strict priority
            # order; DMA trigger instructions cost ~650ns each on the
            # issuing engine, so order = arrival order. Consts ride the
            # gpsimd queue (tiny, lands first).
            wall_sb = consts.tile([P, DCH, 2 * P + KD], bf)
            nc.sync.dma_start(out=wall_sb, in_=wall[:])
            cst_sb = consts.tile([P, P + KD + 2], bf)
            nc.gpsimd.dma_start(out=cst_sb, in_=cst[:])
            pqk_sb = persist.tile([P, 2, S], bf)

            msk_sb = cst_sb[:, 0:P]
            mov_sb = cst_sb[0 : KD + 1, P : P + KD + 2]
            wq_sb = wall_sb[:, :, 0:P]
            wk_sb = wall_sb[:, :, P : 2 * P]
            wv_sb = wall_sb[:, :, 2 * P : 2 * P + KD]

            # piece-0 x^T in two halves so projections start sooner; pos
            # tables trail their piece (the pos-dependent work is cheap)
            xT_sb = persist.tile([P, NCH, DCH, NSL], bf)
            nc.sync.dma_start(out=xT_sb[:, 0, 0:2], in_=xtr[0, :, 0:2])
            nc.sync.dma_start(out=xT_sb[:, 0, 2:4], in_=xtr[0, :, 2:4])
            nc.sync.dma_start(out=pqk_sb[:, :, 0:NSL], in_=pqk[:, :, 0:NSL])
            for n in range(1, NCH):
                sl = slice(n * NSL, (n + 1) * NSL)
                nc.sync.dma_start(out=xT_sb[:, n], in_=xtr[n])
                nc.sync.dma_start(out=pqk_sb[:, :, sl], in_=pqk[:, :, sl])

            # junk matmuls: hold the PE busy (HAM ramp) while DMA lands
            zjunk = consts.tile([P, 2 * P], bf)
            nc.vector.memset(zjunk, 0.0)
            ps_dum = ps_proj.tile([P, 2 * P], f32, tag="ps", name="ps_dum")
            for _ in range(NWARM):
                nc.tensor.matmul(
                    ps_dum, zjunk[:, 0:P], zjunk, start=True, stop=True
                )

            # qT padded by one block so every sT matmul is a uniform N=256
            qT_sb = persist.tile([P, S + P], bf)
            kT_sb = persist.tile([P, S], bf)
            nc.vector.memset(qT_sb[:, S : S + P], 0.0)

            # vT rows 0:64 = raw v projection; row 64 = ones (rowsum rider)
            vT_sb = persist.tile([KD + 1, S], bf)
            nc.vector.memset(vT_sb[KD : KD + 1, :], 1.0)

            # v_aug[key, block, 0:64] = v + bv; col 64 = 1.0; col 65 pad
            v_aug = persist.tile([P, NB, KD + 2], bf)

            out_all = persist.tile([P, NB, KD], f32)
            recip_sb = persist.tile([P, NB, 1], f32)
            out_r = out.rearrange("(q r) k -> r q k", r=P)

            def proj_piece(n):
                sl = slice(n * NSL, (n + 1) * NSL)
                for grp in range(3):  # 0=q, 1=k, 2=v
                    w_g = (wq_sb, wk_sb, wv_sb)[grp]
                    m = P if grp < 2 else KD
                    ps = ps_proj.tile([m, NSL], f32, tag="ps", name="ps")
                    for c in range(DCH):
                        nc.tensor.matmul(
                            ps,
                            w_g[:, c, :m],
                            xT_sb[:, n, c, :],
                            start=(c == 0),
                            stop=(c == DCH - 1),
                        )
                    if grp == 0:
                        nc.vector.tensor_add(qT_sb[:, sl], ps, pqk_sb[:, 0, sl])
                    elif grp == 1:
                        nc.vector.tensor_add(kT_sb[:, sl], ps, pqk_sb[:, 1, sl])
                    else:
                        nc.scalar.copy(vT_sb[0:KD, sl], ps)

            def vgen(t):
                # [128 keys, 66] = vT_aug_block.T @ [I64;bv^T|ones-col]
                g = t // 4
                if t % 4 == 0:
                    v_group[g] = ps_small.tile(
                        [P, 4, KD + 2], f32, tag="small", name="vp4"
                    )
                nc.tensor.matmul(
                    v_group[g][:, t % 4, :], vT_sb[:, ts(t, P)], mov_sb,
                    start=True, stop=True,
                )
                if t % 4 == 3:
                    nc.vector.tensor_copy(
                        v_aug[:, 4 * g : 4 * g + 4, :], v_group.pop(g)
                    )

            p_tiles = {}
            o_group = {}
            v_group = {}

            def score_pair(kbs):
                # paired blocks share one PSUM tile / one exp / one affine /
                # one mask-mul, halving the fixed per-op overheads that
                # otherwise pace the score->exp->mask->attend chain
                s_ps = ps_s.tile([P, 2, 2 * P], f32, tag="s", name="s_ps")
                for j, kb in enumerate(kbs):
                    nc.tensor.matmul(
                        s_ps[:, j, :],
                        kT_sb[:, ts(kb, P)],
                        qT_sb[:, kb * P : kb * P + 2 * P],
                        start=True, stop=True,
                    )
                nb = len(kbs)
                p_sb = work.tile([P, 2, 2, P], bf, tag="p_sb", name="p_sb")
                nc.scalar.activation(
                    out=p_sb[:, 0:nb],
                    in_=s_ps[:, 0:nb].rearrange("c b (h r) -> c b h r", h=2),
                    func=mybir.ActivationFunctionType.Exp,
                )
                # band+causal: halves 0 keep keys c <= r (diag blocks) on
                # gpsimd, halves 1 keep c >= r (off-diag) on DVE/gpsimd
                nc.gpsimd.affine_select(
                    out=p_sb[:, 0:nb, 0, :], in_=p_sb[:, 0:nb, 0, :],
                    pattern=[[0, nb], [1, P]],
                    compare_op=mybir.AluOpType.is_ge,
                    fill=0.0, base=0, channel_multiplier=-1,
                )
                eng = nc.vector if kbs[0] % 4 < 2 else nc.gpsimd
                eng.tensor_mul(
                    p_sb[:, 0:nb, 1, :], p_sb[:, 0:nb, 1, :],
                    msk_sb.rearrange("c (b r) -> c b r", b=1).to_broadcast(
                        (P, nb, P)
                    ),
                )
                for j, kb in enumerate(kbs):
                    p_tiles[kb] = p_sb[:, j]

            def attend(qb):
                g = qb // 4
                if qb % 4 == 0:
                    o_group[g] = ps_small.tile(
                        [P, 4, KD + 2], f32, tag="og", name="o4", bufs=2
                    )
                o_ps = o_group[g][:, qb % 4, :]
                halves = [(p_tiles[qb], 0, qb)]
                if qb > 0:
                    halves.insert(0, (p_tiles[qb - 1], 1, qb - 1))
                for i, (pt, h, kb2) in enumerate(halves):
                    nc.tensor.matmul(
                        o_ps,
                        pt[:, h, :],
                        v_aug[:, kb2, :],
                        start=(i == 0),
                        stop=(i == len(halves) - 1),
                    )
                if qb > 0:
                    p_tiles.pop(qb - 1, None)
                g4 = NB // 4 - 1
                if g == g4 and qb % 2 == 1:
                    # final group: normalize + DMA per 2 blocks on separate
                    # queues so the last transfer is small and completions
                    # overlap the teardown
                    h = (qb - 1) // 2 - 2 * g4  # 0 or 1 within final group
                    o4 = o_group[g]
                    sl2 = slice(4 * g + 2 * h, 4 * g + 2 * h + 2)
                    r2 = recip_sb[:, sl2, :]
                    nc.vector.reciprocal(r2, o4[:, 2 * h : 2 * h + 2, KD : KD + 1])
                    nc.vector.tensor_mul(
                        out_all[:, sl2, :],
                        o4[:, 2 * h : 2 * h + 2, 0:KD],
                        r2.to_broadcast((P, 2, KD)),
                    )
                    eng = nc.sync if h == 0 else nc.scalar
                    eng.dma_start(out=out_r[:, sl2, :], in_=out_all[:, sl2, :])
                    if qb == NB - 1:
                        o_group.pop(g)
                elif qb % 4 == 3:
                    # batched normalize of the 4-block group, then DMA out
                    o4 = o_group.pop(g)
                    r4 = recip_sb[:, 4 * g : 4 * g + 4, :]
                    nc.vector.reciprocal(r4, o4[:, :, KD : KD + 1])
                    nc.vector.tensor_mul(
                        out_all[:, 4 * g : 4 * g + 4, :],
                        o4[:, :, 0:KD],
                        r4.to_broadcast((P, 4, KD)),
                    )
                    nc.sync.dma_start(
                        out=out_r[:, 4 * g : 4 * g + 4, :],
                        in_=out_all[:, 4 * g : 4 * g + 4, :],
                    )

            def junk(k):
                # filler matmuls at known DMA-wait points: keep the HAM
                # busy-streak alive without delaying real work much
                for _ in range(k):
                    nc.tensor.matmul(
                        ps_dum, zjunk[:, 0:P], zjunk, start=True, stop=True
                    )

            # ---- software-pipelined schedule over the 4 column pieces
            # score pairs: {0}, {1,2}, {3,4}, ..., {13,14}, {15}
            pairs = [[0]] + [[2 * g - 1, 2 * g] for g in range(1, NB // 2)] + [[NB - 1]]
            scored = 0   # next pair index
            attended = 0
            for n in range(NCH):
                proj_piece(n)
                hi = 2 * n + 1 if n < NCH - 1 else len(pairs) - 1
                score_pair(pairs[scored]); scored += 1
                for t in range(4 * n, 4 * (n + 1)):
                    vgen(t)
                while scored <= hi:
                    done = pairs[scored - 1][-1]  # blocks masked so far
                    while done - attended > 2:
                        attend(attended)
                        attended += 1
                    score_pair(pairs[scored]); scored += 1
            while attended < NB:
                attend(attended)
                attended += 1

    nc.finalize()
    return nc


def _prep_core_inputs(inputs):
    import ml_dtypes

    bfnp = ml_dtypes.bfloat16
    g = lambda k: np.asarray(inputs[k], dtype=np.float32)
    x = g("x")
    scale = 1.0 / np.sqrt(np.float32(KD))
    temp = float(np.asarray(inputs["temperature"]).reshape(-1)[0])
    alpha = scale * temp  # folded (softmax temp) * (score scale)

    wq = np.concatenate([g("Wqr"), g("Wqi")], axis=1) * (scale * alpha)
    pq = np.concatenate(
        [
            g("pos_qr") * alpha + g("bqr") * (scale * alpha),
            g("pos_qi") * alpha + g("bqi") * (scale * alpha),
        ],
        axis=1,
    ).T  # [128, S]
    wk = np.concatenate([g("Wkr"), -g("Wki")], axis=1)
    pk = np.concatenate(
        [g("pos_kr") + g("bkr"), -(g("pos_ki") + g("bki"))], axis=1
    ).T
    wv = g("Wv")
    bv = g("bv").reshape(KD)

    # merged weights [128, DCH, 320]: [wq | wk | wv]
    pe_pack = lambda w: np.ascontiguousarray(
        w.reshape(DCH, P, w.shape[1]).transpose(1, 0, 2)
    )
    wall = np.concatenate(
        [pe_pack(wq), pe_pack(wk), pe_pack(wv)], axis=2
    )

    # merged consts [128, 128+66]: off-diag mask | vgen moving const
    cc, rr = np.meshgrid(np.arange(P), np.arange(P), indexing="ij")
    cst = np.zeros((P, P + KD + 2), dtype=np.float32)
    cst[:, :P] = (cc >= rr).astype(np.float32)
    cst[:KD, P : P + KD] = np.eye(KD)
    cst[KD, P : P + KD] = bv
    cst[KD, P + KD] = 1.0

    shared = {
        "wall": np.ascontiguousarray(wall).astype(bfnp),
        "pqk": np.ascontiguousarray(np.stack([pq, pk], axis=1)).astype(bfnp),
        "cst": cst.astype(bfnp),
    }
    in_maps = []
    for b in range(NCORES):
        m = dict(shared)
        # xtr[n, p, c, j] = x[b].T[c*128+p, n*512+j]
        xT_b = np.ascontiguousarray(x[b].T)
        m["xtr"] = np.ascontiguousarray(
            xT_b.reshape(DCH, P, NCH, NSL).transpose(2, 1, 0, 3)
        ).astype(bfnp)
        in_maps.append(m)
    return in_maps


def kernel(**inputs):
    from concourse.bass_utils import run_bass_kernel_spmd

    nc = _CACHE.get("nc")
    if nc is None:
        nc = _CACHE["nc"] = _build_nc()
    in_maps = _prep_core_inputs(inputs)
    res = run_bass_kernel_spmd(
        nc, in_maps, core_ids=list(range(NCORES)), **TRACE_KWARGS
    )
    _CACHE["last_result"] = res
    return np.stack([res.results[b]["out"] for b in range(NCORES)], axis=0)
